# revision 8
# baseline (speedup 1.0000x reference)
"""Trainium2 Bass kernel for nn_BFeatVanillaGAT.

Strategy (per sharding hint): edges sharded 8 ways across NeuronCores with
replicated weights. Device computes, for its 8064-edge shard, in
feature-major (transposed) layout:
  - ef_new = relu( relu([x_i, ef, x_j] @ e1_w + e1_b) @ e2_w + e2_b )
  - val/qh/eh projections (pv/pq/pe) with head-major column permutation
  - per-head attention MLP (c1 -> relu -> c2 -> softmax over feature dim)
  - msg = prob * val
Host (numpy) computes the small node-level front half (N=1024 masked
self-attention + LayerNorms), gathers x[src]/x[dst], and the back half
(segment max + p1/p2 MLP).
"""

import sys
import numpy as np

for _p in ("/opt/trn_rl_repo",):
    if _p not in sys.path:
        sys.path.insert(0, _p)

N_CORES = 8
N = 1024
E_TOT = 64512
E_LOC = E_TOT // N_CORES          # 8064
E_BLK = 384
N_BLK = E_LOC // E_BLK            # 21
D = 512
H = 8
DV = 64

# column permutation: natural feature o = dk*8 + h  ->  head-major f = h*64 + dk
_PERM = (np.arange(D) % DV) * H + (np.arange(D) // DV)      # f -> o
_INV = (np.arange(D) % H) * DV + (np.arange(D) // H)        # o -> f


def _relu(x):
    return np.maximum(x, np.float32(0.0))


def _ln(x, g, b, eps=1e-5):
    m = x.mean(-1, keepdims=True, dtype=np.float32)
    v = ((x - m) ** 2).mean(-1, keepdims=True, dtype=np.float32)
    return ((x - m) / np.sqrt(v + np.float32(eps)) * g + b).astype(np.float32)


def _softmax(x, axis=-1):
    x = x - x.max(axis=axis, keepdims=True)
    e = np.exp(x)
    return e / e.sum(axis=axis, keepdims=True, dtype=np.float32)


def _host_front(obj, center, batch_ids, p):
    f32 = np.float32
    mask = batch_ids[:, None] == batch_ids[None, :]

    d = (center[None, :, :] - center[:, None, :]).astype(f32)
    dist = np.sqrt((d * d).sum(-1, keepdims=True, dtype=f32))
    w = np.concatenate([d, dist], -1)                        # (N, N, 4)
    h = _ln(_relu(w @ p["f1_w"] + p["f1_b"]), p["f1_g"], p["f1_beta"])
    h = _ln(_relu(h @ p["f2_w"] + p["f2_b"]), p["f2_g"], p["f2_beta"])
    dw = h @ p["f3_w"] + p["f3_b"]                           # (N, N, H)
    dw = np.where(mask[..., None], dw, f32(0.0))

    q = (obj @ p["q_w"] + p["q_b"]).reshape(N, H, DV)
    k = (obj @ p["k_w"] + p["k_b"]).reshape(N, H, DV)
    v = (obj @ p["v_w"] + p["v_b"]).reshape(N, H, DV)
    att = np.einsum("nhd,mhd->hnm", q, k, dtype=f32) / f32(np.sqrt(DV))
    att = att + dw.transpose(2, 0, 1)
    att = np.where(mask[None], att, f32(-1e9))
    att = _softmax(att, axis=-1)
    out = np.einsum("hnm,mhd->nhd", att, v, dtype=f32).reshape(N, H * DV)
    x = _ln(obj + out @ p["o_w"] + p["o_b"], p["ln_g"], p["ln_b"])
    return x.astype(f32)


def _segment_max(msg, src, n):
    order = np.argsort(src, kind="stable")
    s = src[order]
    ms = msg[order]
    starts = np.flatnonzero(np.r_[True, s[1:] != s[:-1]])
    red = np.maximum.reduceat(ms, starts, axis=0)
    agg = np.zeros((n, msg.shape[1]), dtype=msg.dtype)
    agg[s[starts]] = red
    return agg


_BASS_CACHE = {}


def _build_bass():
    if "nc" in _BASS_CACHE:
        return _BASS_CACHE["nc"]
    import concourse.bacc as bacc
    import concourse.bass as bass
    import concourse.mybir as mybir
    import concourse.tile as tile

    F32 = mybir.dt.float32
    RELU = mybir.ActivationFunctionType.Relu
    IDENT = mybir.ActivationFunctionType.Identity
    EXP = mybir.ActivationFunctionType.Exp

    nc = bacc.Bacc("TRN2", target_bir_lowering=False, debug=False,
                   num_devices=N_CORES)

    dt_in = {}
    for nm, shp in [
        ("xiT", [D, E_LOC]), ("xjT", [D, E_LOC]), ("efT", [D, E_LOC]),
        ("e1w", [3 * D, 2 * D]), ("e1b", [2 * D, 1]),
        ("e2w", [2 * D, D]), ("e2b", [D, 1]),
        ("pvw", [D, D]), ("pvb", [D, 1]),
        ("pqw", [D, D]), ("pqb", [D, 1]),
        ("pew", [D, D]), ("peb", [D, 1]),
        ("c1wT", [128, 128]), ("c1b", [128, 1]),
        ("c2wT", [128, 64]), ("c2b", [64, 1]),
    ]:
        dt_in[nm] = nc.dram_tensor(nm, shp, F32, kind="ExternalInput")
    efoutT_d = nc.dram_tensor("efoutT", [D, E_LOC], F32, kind="ExternalOutput")
    msgT_d = nc.dram_tensor("msgT", [D, E_LOC], F32, kind="ExternalOutput")

    with tile.TileContext(nc) as tc:
        from contextlib import ExitStack
        with ExitStack() as ctx:
            wp = ctx.enter_context(tc.tile_pool(name="w", bufs=1))
            io = ctx.enter_context(tc.tile_pool(name="io", bufs=2))
            mid = ctx.enter_context(tc.tile_pool(name="mid", bufs=1))
            sm = ctx.enter_context(tc.tile_pool(name="sm", bufs=2))
            ps = ctx.enter_context(
                tc.tile_pool(name="ps", bufs=8, space=bass.MemorySpace.PSUM))

            # ---- resident weights/biases ----
            def wtile(nm, pslice, shape, tag):
                t = wp.tile(shape, F32, tag=tag, name=tag)
                nc.sync.dma_start(t[:], dt_in[nm][pslice])
                return t

            e1w_t = [wtile("e1w", np.s_[k * 128:(k + 1) * 128, :],
                           [128, 2 * D], f"e1w{k}") for k in range(12)]
            e2w_t = [wtile("e2w", np.s_[k * 128:(k + 1) * 128, :],
                           [128, D], f"e2w{k}") for k in range(8)]
            pvw_t = [wtile("pvw", np.s_[k * 128:(k + 1) * 128, :],
                           [128, D], f"pvw{k}") for k in range(4)]
            pqw_t = [wtile("pqw", np.s_[k * 128:(k + 1) * 128, :],
                           [128, D], f"pqw{k}") for k in range(4)]
            pew_t = [wtile("pew", np.s_[k * 128:(k + 1) * 128, :],
                           [128, D], f"pew{k}") for k in range(4)]
            c1wTa_t = wtile("c1wT", np.s_[0:64, :], [64, 128], "c1wTa")
            c1wTb_t = wtile("c1wT", np.s_[64:128, :], [64, 128], "c1wTb")
            c2wT_t = wtile("c2wT", np.s_[:, :], [128, 64], "c2wT")

            e1b_t = [wtile("e1b", np.s_[o * 128:(o + 1) * 128, :],
                           [128, 1], f"e1b{o}") for o in range(8)]
            e2b_t = [wtile("e2b", np.s_[o * 128:(o + 1) * 128, :],
                           [128, 1], f"e2b{o}") for o in range(4)]
            pvb_t = [wtile("pvb", np.s_[o * 128:(o + 1) * 128, :],
                           [128, 1], f"pvb{o}") for o in range(4)]
            pqb_t = [wtile("pqb", np.s_[o * 128:(o + 1) * 128, :],
                           [128, 1], f"pqb{o}") for o in range(4)]
            peb_t = [wtile("peb", np.s_[o * 128:(o + 1) * 128, :],
                           [128, 1], f"peb{o}") for o in range(4)]
            c1b_t = wtile("c1b", np.s_[:, :], [128, 1], "c1b")
            c2b_t = wtile("c2b", np.s_[:, :], [64, 1], "c2b")

            ones64 = wp.tile([64, 1], F32, tag="ones64", name="ones64")
            nc.vector.memset(ones64[:], 1.0)
            ones1 = wp.tile([1, 64], F32, tag="ones1", name="ones1")
            nc.vector.memset(ones1[:], 1.0)

            for b in range(N_BLK):
                eb = np.s_[b * E_BLK:(b + 1) * E_BLK]

                xi_t = [io.tile([128, E_BLK], F32, tag=f"xi{k}", name=f"xi{k}") for k in range(4)]
                xj_t = [io.tile([128, E_BLK], F32, tag=f"xj{k}", name=f"xj{k}") for k in range(4)]
                ef_t = [io.tile([128, E_BLK], F32, tag=f"ef{k}", name=f"ef{k}") for k in range(4)]
                for k in range(4):
                    nc.sync.dma_start(xi_t[k][:], dt_in["xiT"][k * 128:(k + 1) * 128, eb])
                    nc.sync.dma_start(xj_t[k][:], dt_in["xjT"][k * 128:(k + 1) * 128, eb])
                    nc.sync.dma_start(ef_t[k][:], dt_in["efT"][k * 128:(k + 1) * 128, eb])

                cat = xi_t + ef_t + xj_t          # K-chunks 0..11 of [x_i, ef, x_j]

                # ---- e1: h1 = relu(cat @ e1_w + e1_b), feature-major ----
                h1_t = []
                for o in range(8):
                    acc = ps.tile([128, E_BLK], F32, tag="ps", name="ps")
                    for k in range(12):
                        nc.tensor.matmul(
                            acc[:], e1w_t[k][:, o * 128:(o + 1) * 128], cat[k][:],
                            start=(k == 0), stop=(k == 11))
                    h = mid.tile([128, E_BLK], F32, tag=f"h1_{o}", name=f"h1_{o}")
                    nc.scalar.activation(h[:], acc[:], RELU, bias=e1b_t[o][:])
                    h1_t.append(h)

                # ---- e2: efout = relu(h1 @ e2_w + e2_b) ----
                for o in range(4):
                    acc = ps.tile([128, E_BLK], F32, tag="ps", name="ps")
                    for k in range(8):
                        nc.tensor.matmul(
                            acc[:], e2w_t[k][:, o * 128:(o + 1) * 128], h1_t[k][:],
                            start=(k == 0), stop=(k == 7))
                    eo = io.tile([128, E_BLK], F32, tag=f"eo{o}", name=f"eo{o}")
                    nc.scalar.activation(eo[:], acc[:], RELU, bias=e2b_t[o][:])
                    nc.sync.dma_start(efoutT_d[o * 128:(o + 1) * 128, eb], eo[:])

                # ---- per-head: projections + attention MLP + softmax + msg ----
                def proj_head(wts, bts, src_t, h, tagp):
                    acc = ps.tile([64, E_BLK], F32, tag="ps", name="ps")
                    for k in range(4):
                        nc.tensor.matmul(
                            acc[:], wts[k][:, h * 64:(h + 1) * 64], src_t[k][:],
                            start=(k == 0), stop=(k == 3))
                    t = mid.tile([64, E_BLK], F32, tag=tagp, name=tagp, bufs=2)
                    nc.scalar.activation(t[:], acc[:], IDENT,
                                         bias=bts[h // 2][(h % 2) * 64:(h % 2) * 64 + 64, :])
                    return t

                for h in range(8):
                    val_h = proj_head(pvw_t, pvb_t, xj_t, h, "val")
                    qh_h = proj_head(pqw_t, pqb_t, xi_t, h, "qh")
                    eh_h = proj_head(pew_t, peb_t, ef_t, h, "eh")

                    m1 = ps.tile([128, E_BLK], F32, tag="ps", name="ps")
                    nc.tensor.matmul(m1[:], c1wTa_t[:], qh_h[:],
                                     start=True, stop=False)
                    nc.tensor.matmul(m1[:], c1wTb_t[:], eh_h[:],
                                     start=False, stop=True)
                    s1 = sm.tile([128, E_BLK], F32, tag="s1", name="s1")
                    nc.scalar.activation(s1[:], m1[:], RELU, bias=c1b_t[:])

                    m2 = ps.tile([64, E_BLK], F32, tag="ps", name="ps")
                    nc.tensor.matmul(m2[:], c2wT_t[:], s1[:], start=True, stop=True)
                    ex = sm.tile([64, E_BLK], F32, tag="ex", name="ex")
                    nc.scalar.activation(ex[:], m2[:], EXP, bias=c2b_t[:])

                    ssum = ps.tile([1, E_BLK], F32, tag="ps", name="ps")
                    nc.tensor.matmul(ssum[:], ones64[:], ex[:], start=True, stop=True)
                    rec = sm.tile([1, E_BLK], F32, tag="rec", name="rec")
                    nc.vector.reciprocal(rec[:], ssum[:])
                    rb = ps.tile([64, E_BLK], F32, tag="ps", name="ps")
                    nc.tensor.matmul(rb[:], ones1[:], rec[:], start=True, stop=True)

                    pr = sm.tile([64, E_BLK], F32, tag="pr", name="pr")
                    nc.vector.tensor_mul(pr[:], ex[:], rb[:])
                    msg_h = io.tile([64, E_BLK], F32, tag="msg", name="msg", bufs=3)
                    nc.vector.tensor_mul(msg_h[:], pr[:], val_h[:])
                    nc.sync.dma_start(msgT_d[h * 64:(h + 1) * 64, eb], msg_h[:])

    nc.compile()
    _BASS_CACHE["nc"] = nc
    return nc


def _run_device(x, ef, src, dst, p):
    from concourse.bass_utils import run_bass_kernel_spmd

    f32 = np.float32
    nc = _build_bass()

    shared = {
        "e1w": np.ascontiguousarray(p["e1_w"], dtype=f32),
        "e1b": np.ascontiguousarray(p["e1_b"], dtype=f32).reshape(-1, 1),
        "e2w": np.ascontiguousarray(p["e2_w"], dtype=f32),
        "e2b": np.ascontiguousarray(p["e2_b"], dtype=f32).reshape(-1, 1),
        "pvw": np.ascontiguousarray(p["pv_w"][:, _PERM], dtype=f32),
        "pvb": np.ascontiguousarray(p["pv_b"][_PERM], dtype=f32).reshape(-1, 1),
        "pqw": np.ascontiguousarray(p["pq_w"][:, _PERM], dtype=f32),
        "pqb": np.ascontiguousarray(p["pq_b"][_PERM], dtype=f32).reshape(-1, 1),
        "pew": np.ascontiguousarray(p["pe_w"][:, _PERM], dtype=f32),
        "peb": np.ascontiguousarray(p["pe_b"][_PERM], dtype=f32).reshape(-1, 1),
        "c1wT": np.ascontiguousarray(p["c1_w"].T, dtype=f32),
        "c1b": np.ascontiguousarray(p["c1_b"], dtype=f32).reshape(-1, 1),
        "c2wT": np.ascontiguousarray(p["c2_w"].T, dtype=f32),
        "c2b": np.ascontiguousarray(p["c2_b"], dtype=f32).reshape(-1, 1),
    }
    in_maps = []
    for c in range(N_CORES):
        sl = np.s_[c * E_LOC:(c + 1) * E_LOC]
        m = dict(shared)
        m["xiT"] = np.ascontiguousarray(x[src[sl]].T, dtype=f32)
        m["xjT"] = np.ascontiguousarray(x[dst[sl]].T, dtype=f32)
        m["efT"] = np.ascontiguousarray(ef[sl].T, dtype=f32)
        in_maps.append(m)

    br = run_bass_kernel_spmd(nc, in_maps, list(range(N_CORES)))
    _BASS_CACHE["last_results"] = br
    res = br.results

    ef_out = np.empty((E_TOT, D), dtype=f32)
    msg = np.empty((E_TOT, D), dtype=f32)
    for c in range(N_CORES):
        sl = np.s_[c * E_LOC:(c + 1) * E_LOC]
        ef_out[sl] = res[c]["efoutT"].T
        msg[sl] = res[c]["msgT"][_INV, :].T
    return ef_out, msg


def kernel(obj_feature_3d, edge_feature_3d, obj_center, edge_index, batch_ids,
           params):
    f32 = np.float32
    obj = np.asarray(obj_feature_3d, dtype=f32)
    ef = np.asarray(edge_feature_3d, dtype=f32)
    center = np.asarray(obj_center, dtype=f32)
    eidx = np.asarray(edge_index)
    bids = np.asarray(batch_ids)
    p = {k: np.asarray(v, dtype=f32) for k, v in params.items()}

    x = _host_front(obj, center, bids, p)
    src, dst = eidx[0], eidx[1]

    ef_out, msg = _run_device(x, ef, src, dst, p)

    agg = _segment_max(msg, src, N)
    xo = _relu(np.concatenate([x, agg], 1) @ p["p1_w"] + p["p1_b"])
    xo = xo @ p["p2_w"] + p["p2_b"]
    return _relu(xo).astype(f32), ef_out


# revision 10
# speedup vs baseline: 3.0092x; 3.0092x over previous
"""Trainium2 Bass kernel for nn_BFeatVanillaGAT.

Strategy (per sharding hint): edges sharded 8 ways across NeuronCores with
replicated weights. Device computes, for its 8064-edge shard, in
feature-major (transposed) layout:
  - ef_new = relu( relu([x_i, ef, x_j] @ e1_w + e1_b) @ e2_w + e2_b )
  - val/qh/eh projections (pv/pq/pe) with head-major column permutation
  - per-head attention MLP (c1 -> relu -> c2 -> softmax over feature dim)
  - msg = prob * val
Host (numpy) computes the small node-level front half (N=1024 masked
self-attention + LayerNorms), gathers x[src]/x[dst], and the back half
(segment max + p1/p2 MLP).
"""

import sys
import numpy as np

for _p in ("/opt/trn_rl_repo",):
    if _p not in sys.path:
        sys.path.insert(0, _p)

N_CORES = 8
N = 1024
E_TOT = 64512
E_LOC = E_TOT // N_CORES          # 8064
E_BLK = 384
N_BLK = E_LOC // E_BLK            # 21
D = 512
H = 8
DV = 64

# column permutation: natural feature o = dk*8 + h  ->  head-major f = h*64 + dk
_PERM = (np.arange(D) % DV) * H + (np.arange(D) // DV)      # f -> o
_INV = (np.arange(D) % H) * DV + (np.arange(D) // H)        # o -> f


def _relu(x):
    return np.maximum(x, np.float32(0.0))


def _ln(x, g, b, eps=1e-5):
    m = x.mean(-1, keepdims=True, dtype=np.float32)
    v = ((x - m) ** 2).mean(-1, keepdims=True, dtype=np.float32)
    return ((x - m) / np.sqrt(v + np.float32(eps)) * g + b).astype(np.float32)


def _softmax(x, axis=-1):
    x = x - x.max(axis=axis, keepdims=True)
    e = np.exp(x)
    return e / e.sum(axis=axis, keepdims=True, dtype=np.float32)


def _host_front(obj, center, batch_ids, p):
    f32 = np.float32
    mask = batch_ids[:, None] == batch_ids[None, :]

    d = (center[None, :, :] - center[:, None, :]).astype(f32)
    dist = np.sqrt((d * d).sum(-1, keepdims=True, dtype=f32))
    w = np.concatenate([d, dist], -1).reshape(N * N, 4)      # (N*N, 4)
    h = _ln(_relu(w @ p["f1_w"] + p["f1_b"]), p["f1_g"], p["f1_beta"])
    h = _ln(_relu(h @ p["f2_w"] + p["f2_b"]), p["f2_g"], p["f2_beta"])
    dw = (h @ p["f3_w"] + p["f3_b"]).reshape(N, N, H)        # (N, N, H)
    dw = np.where(mask[..., None], dw, f32(0.0))

    q = (obj @ p["q_w"] + p["q_b"]).reshape(N, H, DV)
    k = (obj @ p["k_w"] + p["k_b"]).reshape(N, H, DV)
    v = (obj @ p["v_w"] + p["v_b"]).reshape(N, H, DV)
    qT = np.ascontiguousarray(q.transpose(1, 0, 2))          # (H, N, DV)
    kT = np.ascontiguousarray(k.transpose(1, 0, 2))
    vT = np.ascontiguousarray(v.transpose(1, 0, 2))
    att = qT @ kT.transpose(0, 2, 1) / f32(np.sqrt(DV))      # (H, N, N)
    att = att + dw.transpose(2, 0, 1)
    att = np.where(mask[None], att, f32(-1e9))
    att = _softmax(att, axis=-1)
    out = (att @ vT).transpose(1, 0, 2).reshape(N, H * DV)   # (N, H*DV)
    x = _ln(obj + out @ p["o_w"] + p["o_b"], p["ln_g"], p["ln_b"])
    return x.astype(f32)


def _segment_max(msg, src, n):
    order = np.argsort(src, kind="stable")
    s = src[order]
    ms = msg[order]
    starts = np.flatnonzero(np.r_[True, s[1:] != s[:-1]])
    red = np.maximum.reduceat(ms, starts, axis=0)
    agg = np.zeros((n, msg.shape[1]), dtype=msg.dtype)
    agg[s[starts]] = red
    return agg


_BASS_CACHE = {}


def _build_bass():
    if "nc" in _BASS_CACHE:
        return _BASS_CACHE["nc"]
    import concourse.bacc as bacc
    import concourse.bass as bass
    import concourse.mybir as mybir
    import concourse.tile as tile

    F32 = mybir.dt.float32
    RELU = mybir.ActivationFunctionType.Relu
    IDENT = mybir.ActivationFunctionType.Identity
    EXP = mybir.ActivationFunctionType.Exp

    nc = bacc.Bacc("TRN2", target_bir_lowering=False, debug=False,
                   num_devices=N_CORES)

    dt_in = {}
    for nm, shp in [
        ("xiT", [D, E_LOC]), ("xjT", [D, E_LOC]), ("efT", [D, E_LOC]),
        ("e1w", [3 * D, 2 * D]), ("e1b", [2 * D, 1]),
        ("e2w", [2 * D, D]), ("e2b", [D, 1]),
        ("pvw", [D, D]), ("pvb", [D, 1]),
        ("pqw", [D, D]), ("pqb", [D, 1]),
        ("pew", [D, D]), ("peb", [D, 1]),
        ("c1wT", [128, 128]), ("c1b", [128, 1]),
        ("c2wT", [128, 64]), ("c2b", [64, 1]),
    ]:
        dt_in[nm] = nc.dram_tensor(nm, shp, F32, kind="ExternalInput")
    efoutT_d = nc.dram_tensor("efoutT", [D, E_LOC], F32, kind="ExternalOutput")
    msgT_d = nc.dram_tensor("msgT", [D, E_LOC], F32, kind="ExternalOutput")

    with tile.TileContext(nc) as tc:
        from contextlib import ExitStack
        with ExitStack() as ctx:
            wp = ctx.enter_context(tc.tile_pool(name="w", bufs=1))
            io = ctx.enter_context(tc.tile_pool(name="io", bufs=2))
            mid = ctx.enter_context(tc.tile_pool(name="mid", bufs=1))
            sm = ctx.enter_context(tc.tile_pool(name="sm", bufs=2))
            ps = ctx.enter_context(
                tc.tile_pool(name="ps", bufs=8, space=bass.MemorySpace.PSUM))

            # ---- resident weights/biases ----
            def wtile(nm, pslice, shape, tag):
                t = wp.tile(shape, F32, tag=tag, name=tag)
                nc.sync.dma_start(t[:], dt_in[nm][pslice])
                return t

            e1w_t = [wtile("e1w", np.s_[k * 128:(k + 1) * 128, :],
                           [128, 2 * D], f"e1w{k}") for k in range(12)]
            e2w_t = [wtile("e2w", np.s_[k * 128:(k + 1) * 128, :],
                           [128, D], f"e2w{k}") for k in range(8)]
            pvw_t = [wtile("pvw", np.s_[k * 128:(k + 1) * 128, :],
                           [128, D], f"pvw{k}") for k in range(4)]
            pqw_t = [wtile("pqw", np.s_[k * 128:(k + 1) * 128, :],
                           [128, D], f"pqw{k}") for k in range(4)]
            pew_t = [wtile("pew", np.s_[k * 128:(k + 1) * 128, :],
                           [128, D], f"pew{k}") for k in range(4)]
            c1wTa_t = wtile("c1wT", np.s_[0:64, :], [64, 128], "c1wTa")
            c1wTb_t = wtile("c1wT", np.s_[64:128, :], [64, 128], "c1wTb")
            c2wT_t = wtile("c2wT", np.s_[:, :], [128, 64], "c2wT")

            e1b_t = [wtile("e1b", np.s_[o * 128:(o + 1) * 128, :],
                           [128, 1], f"e1b{o}") for o in range(8)]
            e2b_t = [wtile("e2b", np.s_[o * 128:(o + 1) * 128, :],
                           [128, 1], f"e2b{o}") for o in range(4)]
            pvb_t = [wtile("pvb", np.s_[o * 128:(o + 1) * 128, :],
                           [128, 1], f"pvb{o}") for o in range(4)]
            pqb_t = [wtile("pqb", np.s_[o * 128:(o + 1) * 128, :],
                           [128, 1], f"pqb{o}") for o in range(4)]
            peb_t = [wtile("peb", np.s_[o * 128:(o + 1) * 128, :],
                           [128, 1], f"peb{o}") for o in range(4)]
            c1b_t = wtile("c1b", np.s_[:, :], [128, 1], "c1b")
            c2b_t = wtile("c2b", np.s_[:, :], [64, 1], "c2b")

            ones64 = wp.tile([64, 1], F32, tag="ones64", name="ones64")
            nc.vector.memset(ones64[:], 1.0)
            ones1 = wp.tile([1, 64], F32, tag="ones1", name="ones1")
            nc.vector.memset(ones1[:], 1.0)

            for b in range(N_BLK):
                eb = np.s_[b * E_BLK:(b + 1) * E_BLK]

                xi_t = [io.tile([128, E_BLK], F32, tag=f"xi{k}", name=f"xi{k}") for k in range(4)]
                xj_t = [io.tile([128, E_BLK], F32, tag=f"xj{k}", name=f"xj{k}") for k in range(4)]
                ef_t = [io.tile([128, E_BLK], F32, tag=f"ef{k}", name=f"ef{k}") for k in range(4)]
                for k in range(4):
                    nc.sync.dma_start(xi_t[k][:], dt_in["xiT"][k * 128:(k + 1) * 128, eb])
                    nc.sync.dma_start(xj_t[k][:], dt_in["xjT"][k * 128:(k + 1) * 128, eb])
                    nc.sync.dma_start(ef_t[k][:], dt_in["efT"][k * 128:(k + 1) * 128, eb])

                cat = xi_t + ef_t + xj_t          # K-chunks 0..11 of [x_i, ef, x_j]

                # ---- e1: h1 = relu(cat @ e1_w + e1_b), feature-major ----
                h1_t = []
                for o in range(8):
                    acc = ps.tile([128, E_BLK], F32, tag="ps", name="ps")
                    for k in range(12):
                        nc.tensor.matmul(
                            acc[:], e1w_t[k][:, o * 128:(o + 1) * 128], cat[k][:],
                            start=(k == 0), stop=(k == 11))
                    h = mid.tile([128, E_BLK], F32, tag=f"h1_{o}", name=f"h1_{o}")
                    nc.scalar.activation(h[:], acc[:], RELU, bias=e1b_t[o][:])
                    h1_t.append(h)

                # ---- e2: efout = relu(h1 @ e2_w + e2_b) ----
                for o in range(4):
                    acc = ps.tile([128, E_BLK], F32, tag="ps", name="ps")
                    for k in range(8):
                        nc.tensor.matmul(
                            acc[:], e2w_t[k][:, o * 128:(o + 1) * 128], h1_t[k][:],
                            start=(k == 0), stop=(k == 7))
                    eo = io.tile([128, E_BLK], F32, tag=f"eo{o}", name=f"eo{o}")
                    nc.scalar.activation(eo[:], acc[:], RELU, bias=e2b_t[o][:])
                    nc.sync.dma_start(efoutT_d[o * 128:(o + 1) * 128, eb], eo[:])

                # ---- per-head: projections + attention MLP + softmax + msg ----
                def proj_head(wts, bts, src_t, h, tagp):
                    acc = ps.tile([64, E_BLK], F32, tag="ps", name="ps")
                    for k in range(4):
                        nc.tensor.matmul(
                            acc[:], wts[k][:, h * 64:(h + 1) * 64], src_t[k][:],
                            start=(k == 0), stop=(k == 3))
                    t = mid.tile([64, E_BLK], F32, tag=tagp, name=tagp, bufs=2)
                    nc.scalar.activation(t[:], acc[:], IDENT,
                                         bias=bts[h // 2][(h % 2) * 64:(h % 2) * 64 + 64, :])
                    return t

                for h in range(8):
                    val_h = proj_head(pvw_t, pvb_t, xj_t, h, "val")
                    qh_h = proj_head(pqw_t, pqb_t, xi_t, h, "qh")
                    eh_h = proj_head(pew_t, peb_t, ef_t, h, "eh")

                    m1 = ps.tile([128, E_BLK], F32, tag="ps", name="ps")
                    nc.tensor.matmul(m1[:], c1wTa_t[:], qh_h[:],
                                     start=True, stop=False)
                    nc.tensor.matmul(m1[:], c1wTb_t[:], eh_h[:],
                                     start=False, stop=True)
                    s1 = sm.tile([128, E_BLK], F32, tag="s1", name="s1")
                    nc.scalar.activation(s1[:], m1[:], RELU, bias=c1b_t[:])

                    m2 = ps.tile([64, E_BLK], F32, tag="ps", name="ps")
                    nc.tensor.matmul(m2[:], c2wT_t[:], s1[:], start=True, stop=True)
                    ex = sm.tile([64, E_BLK], F32, tag="ex", name="ex")
                    nc.scalar.activation(ex[:], m2[:], EXP, bias=c2b_t[:])

                    ssum = ps.tile([1, E_BLK], F32, tag="ps", name="ps")
                    nc.tensor.matmul(ssum[:], ones64[:], ex[:], start=True, stop=True)
                    rec = sm.tile([1, E_BLK], F32, tag="rec", name="rec")
                    nc.vector.reciprocal(rec[:], ssum[:])
                    rb = ps.tile([64, E_BLK], F32, tag="ps", name="ps")
                    nc.tensor.matmul(rb[:], ones1[:], rec[:], start=True, stop=True)

                    pr = sm.tile([64, E_BLK], F32, tag="pr", name="pr")
                    nc.vector.tensor_mul(pr[:], ex[:], rb[:])
                    msg_h = io.tile([64, E_BLK], F32, tag="msg", name="msg", bufs=3)
                    nc.vector.tensor_mul(msg_h[:], pr[:], val_h[:])
                    nc.sync.dma_start(msgT_d[h * 64:(h + 1) * 64, eb], msg_h[:])

    nc.compile()
    _BASS_CACHE["nc"] = nc
    return nc


def _run_device(x, ef, src, dst, p):
    import os
    os.environ["BASS_NEVER_TRACE"] = "1"   # no NTFF hook in this container
    from concourse.bass_utils import run_bass_kernel_spmd

    f32 = np.float32
    nc = _build_bass()

    shared = {
        "e1w": np.ascontiguousarray(p["e1_w"], dtype=f32),
        "e1b": np.ascontiguousarray(p["e1_b"], dtype=f32).reshape(-1, 1),
        "e2w": np.ascontiguousarray(p["e2_w"], dtype=f32),
        "e2b": np.ascontiguousarray(p["e2_b"], dtype=f32).reshape(-1, 1),
        "pvw": np.ascontiguousarray(p["pv_w"][:, _PERM], dtype=f32),
        "pvb": np.ascontiguousarray(p["pv_b"][_PERM], dtype=f32).reshape(-1, 1),
        "pqw": np.ascontiguousarray(p["pq_w"][:, _PERM], dtype=f32),
        "pqb": np.ascontiguousarray(p["pq_b"][_PERM], dtype=f32).reshape(-1, 1),
        "pew": np.ascontiguousarray(p["pe_w"][:, _PERM], dtype=f32),
        "peb": np.ascontiguousarray(p["pe_b"][_PERM], dtype=f32).reshape(-1, 1),
        "c1wT": np.ascontiguousarray(p["c1_w"].T, dtype=f32),
        "c1b": np.ascontiguousarray(p["c1_b"], dtype=f32).reshape(-1, 1),
        "c2wT": np.ascontiguousarray(p["c2_w"].T, dtype=f32),
        "c2b": np.ascontiguousarray(p["c2_b"], dtype=f32).reshape(-1, 1),
    }
    in_maps = []
    for c in range(N_CORES):
        sl = np.s_[c * E_LOC:(c + 1) * E_LOC]
        m = dict(shared)
        m["xiT"] = np.ascontiguousarray(x[src[sl]].T, dtype=f32)
        m["xjT"] = np.ascontiguousarray(x[dst[sl]].T, dtype=f32)
        m["efT"] = np.ascontiguousarray(ef[sl].T, dtype=f32)
        in_maps.append(m)

    br = run_bass_kernel_spmd(nc, in_maps, list(range(N_CORES)))
    _BASS_CACHE["last_results"] = br
    res = br.results

    ef_out = np.empty((E_TOT, D), dtype=f32)
    msg = np.empty((E_TOT, D), dtype=f32)
    for c in range(N_CORES):
        sl = np.s_[c * E_LOC:(c + 1) * E_LOC]
        ef_out[sl] = res[c]["efoutT"].T
        msg[sl] = res[c]["msgT"][_INV, :].T
    return ef_out, msg


def kernel(obj_feature_3d, edge_feature_3d, obj_center, edge_index, batch_ids,
           params):
    f32 = np.float32
    obj = np.asarray(obj_feature_3d, dtype=f32)
    ef = np.asarray(edge_feature_3d, dtype=f32)
    center = np.asarray(obj_center, dtype=f32)
    eidx = np.asarray(edge_index)
    bids = np.asarray(batch_ids)
    p = {k: np.asarray(v, dtype=f32) for k, v in params.items()}

    x = _host_front(obj, center, bids, p)
    src, dst = eidx[0], eidx[1]

    ef_out, msg = _run_device(x, ef, src, dst, p)

    agg = _segment_max(msg, src, N)
    xo = _relu(np.concatenate([x, agg], 1) @ p["p1_w"] + p["p1_b"])
    xo = xo @ p["p2_w"] + p["p2_b"]
    return _relu(xo).astype(f32), ef_out
